# revision 13
# baseline (speedup 1.0000x reference)
"""Causal self-attention (B=2, T=2048, D=1024, H=16) on 8 TRN2 NeuronCores.

Sharding: 8-way tensor-parallel over heads (2 heads/core, both batches),
then one 8-core AllToAll reshards from head-channels to token-slices so each
core computes a disjoint [512, 1024] slice of the output projection.

Per-core program (SPMD, identical program, per-core data):
  core r: heads {2r, 2r+1}  -> qkv channel slice [128r : 128r+128)
          output slice      -> batch r//4, tokens [512*(r%4), 512*(r%4)+512)

bf16 matmul inputs (host-converted), fp32 PSUM accumulation, fp32 output.
Numpy-simulated end-to-end max rel err vs the fp32 reference: ~3.3e-3.
"""

import numpy as np
import ml_dtypes
from contextlib import ExitStack

import concourse.bass as bass
import concourse.tile as tile
from concourse import mybir, bacc
from concourse.bass_utils import run_bass_kernel_spmd

F32 = mybir.dt.float32
BF16 = mybir.dt.bfloat16

B, T, D, H, HD = 2, 2048, 1024, 16, 64
NC = 8  # cores
TI = B * T  # token instances = 4096
SCALE = HD ** -0.5


def build_nc() -> bass.Bass:
    nc = bacc.Bacc("TRN2", target_bir_lowering=False, debug=False, num_devices=NC)

    xf = nc.dram_tensor("xf", [TI, D], BF16, kind="ExternalInput").ap()
    wq = nc.dram_tensor("wq", [D, 128], BF16, kind="ExternalInput").ap()
    wk = nc.dram_tensor("wk", [D, 128], BF16, kind="ExternalInput").ap()
    wv = nc.dram_tensor("wv", [D, 128], BF16, kind="ExternalInput").ap()
    bq = nc.dram_tensor("bq", [128], BF16, kind="ExternalInput").ap()
    bk = nc.dram_tensor("bk", [128], BF16, kind="ExternalInput").ap()
    bv = nc.dram_tensor("bv", [128], BF16, kind="ExternalInput").ap()
    wo = nc.dram_tensor("wo", [D, D], BF16, kind="ExternalInput").ap()
    bo = nc.dram_tensor("bo", [D], BF16, kind="ExternalInput").ap()
    tri = nc.dram_tensor("tri", [128, 128], BF16, kind="ExternalInput").ap()  # A^T: 0 on/below diag, -240 above
    eye = nc.dram_tensor("eye", [128, 128], BF16, kind="ExternalInput").ap()
    out = nc.dram_tensor("out", [512, D], F32, kind="ExternalOutput").ap()

    with tile.TileContext(nc) as tc, ExitStack() as ctx:
        const = ctx.enter_context(tc.tile_pool(name="const", bufs=1))
        qkvp = ctx.enter_context(tc.tile_pool(name="qkvp", bufs=1))
        xload = ctx.enter_context(tc.tile_pool(name="xload", bufs=5))
        xtp = ctx.enter_context(tc.tile_pool(name="xtp", bufs=2))
        vtb = ctx.enter_context(tc.tile_pool(name="vtb", bufs=2))
        ptp = ctx.enter_context(tc.tile_pool(name="ptp", bufs=4))
        rp = ctx.enter_context(tc.tile_pool(name="rp", bufs=2))
        atp = ctx.enter_context(tc.tile_pool(name="atp", bufs=3))
        aoutp = ctx.enter_context(tc.tile_pool(name="aoutp", bufs=1))
        osb = ctx.enter_context(tc.tile_pool(name="osb", bufs=2))
        psA = ctx.enter_context(tc.tile_pool(name="psA", bufs=5, space="PSUM"))
        psB = ctx.enter_context(tc.tile_pool(name="psB", bufs=3, space="PSUM"))
        dram = ctx.enter_context(tc.tile_pool(name="dram", bufs=1, space="DRAM"))

        # ---- constants / weights -------------------------------------------------
        wq_sb = const.tile([128, D], BF16)  # col 128c+m  <- wq[128c+p, m]
        wk_sb = const.tile([128, D], BF16)
        wv_sb = const.tile([128, D], BF16)
        nc.sync.dma_start(
            wq_sb[:].rearrange("p (c m) -> p c m", c=8),
            wq.rearrange("(c p) m -> p c m", p=128),
        )
        nc.sync.dma_start(
            wk_sb[:].rearrange("p (c m) -> p c m", c=8),
            wk.rearrange("(c p) m -> p c m", p=128),
        )
        nc.sync.dma_start(
            wv_sb[:].rearrange("p (c m) -> p c m", c=8),
            wv.rearrange("(c p) m -> p c m", p=128),
        )
        wo_sb = const.tile([128, 8 * D], BF16)  # col 1024c+n <- wo[128c+p, n]
        nc.sync.dma_start(
            wo_sb[:].rearrange("p (c n) -> p c n", c=8),
            wo.rearrange("(c p) n -> p c n", p=128),
        )
        bq_sb = const.tile([1, 128], BF16)
        bk_sb = const.tile([1, 128], BF16)
        bv_sb = const.tile([1, 128], BF16)
        bo_sb = const.tile([1, D], BF16)
        nc.sync.dma_start(bq_sb[:], bq[None, :])
        nc.sync.dma_start(bk_sb[:], bk[None, :])
        nc.sync.dma_start(bv_sb[:], bv[None, :])
        nc.sync.dma_start(bo_sb[:], bo[None, :])
        tri_sb = const.tile([128, 128], BF16)
        eye_sb = const.tile([128, 128], BF16)
        nc.sync.dma_start(tri_sb[:], tri[:])
        nc.sync.dma_start(eye_sb[:], eye[:])
        ones_sb = const.tile([1, 512], BF16)
        nc.vector.memset(ones_sb[:], 1.0)

        # Q^T / K^T, channels(128) x token-instances(4096)
        qt_sb = qkvp.tile([128, TI], BF16)
        kt_sb = qkvp.tile([128, TI], BF16)
        # V' : [kpos(128), 32 ktiles x (2 heads x 65)]; col 130*kt + 65*h + d,
        # d==64 is the ones column (softmax denominator trick)
        vp_sb = qkvp.tile([128, 32 * 130], BF16)
        vp_ones = vp_sb.rearrange("p (kt h d) -> p kt h d", kt=32, h=2, d=65)[
            :, :, :, 64:65
        ]
        nc.vector.memset(vp_ones, 1.0)

        a2a_in = dram.tile([1024, 512], BF16)
        a2a_out = dram.tile([1024, 512], BF16)

        # ---- phase A/B: x^T then QKV projections, per 512-token block ------------
        for b in range(B):
            for blk in range(4):
                base = 2048 * b + 512 * blk
                xts = []
                for i in range(4):
                    x_t = xload.tile([128, D], BF16, name="x_t")
                    nc.sync.dma_start(x_t[:], xf[base + 128 * i : base + 128 * (i + 1), :])
                    xts.append(x_t)
                xT = xtp.tile([128, 8 * 512], BF16)  # col 512c + t
                for c in range(8):
                    ps = psA.tile([128, 512], BF16, name="ps_t", tag="ps")
                    for i in range(4):
                        nc.tensor.transpose(
                            ps[:, 128 * i : 128 * (i + 1)],
                            xts[i][:, 128 * c : 128 * (c + 1)],
                            eye_sb[:],
                        )
                    eng = nc.vector if c % 2 == 0 else nc.scalar
                    if eng is nc.vector:
                        eng.tensor_copy(xT[:, 512 * c : 512 * (c + 1)], ps[:])
                    else:
                        eng.copy(xT[:, 512 * c : 512 * (c + 1)], ps[:])

                # projections: psum[128 ch, 512 tok] accumulated over 8 e-chunks
                for w_sb, b_sb, which in (
                    (wq_sb, bq_sb, "q"),
                    (wk_sb, bk_sb, "k"),
                    (wv_sb, bv_sb, "v"),
                ):
                    ps = psA.tile([128, 512], F32, name="ps_p", tag="ps")
                    for c in range(8):
                        nc.tensor.matmul(
                            ps[:],
                            w_sb[:, 128 * c : 128 * (c + 1)],
                            xT[:, 512 * c : 512 * (c + 1)],
                            start=(c == 0),
                            stop=False,
                        )
                    nc.tensor.matmul(
                        ps[:], b_sb[:], ones_sb[:], start=False, stop=True
                    )
                    if which == "q":
                        nc.scalar.copy(qt_sb[:, base : base + 512], ps[:])
                    elif which == "k":
                        nc.scalar.copy(kt_sb[:, base : base + 512], ps[:])
                    else:
                        vt_blk = vtb.tile([128, 512], BF16)
                        nc.vector.tensor_copy(vt_blk[:], ps[:])
                        # V'[tok, ch] tiles via PE transpose
                        ps2 = psA.tile([128, 512], BF16, name="ps_vt", tag="ps")
                        for i in range(4):
                            nc.tensor.transpose(
                                ps2[:, 128 * i : 128 * (i + 1)],
                                vt_blk[:, 128 * i : 128 * (i + 1)],
                                eye_sb[:],
                            )
                        kt0 = 16 * b + 4 * blk
                        dst = vp_sb[:, 130 * kt0 : 130 * (kt0 + 4)].rearrange(
                            "p (kt h d) -> p kt h d", kt=4, h=2, d=65
                        )[:, :, :, :64]
                        src = ps2.rearrange("p (i h d) -> p i h d", i=4, h=2, d=64)
                        nc.vector.tensor_copy(dst, src)

        # ---- phase C: attention, per (batch, 512-query-block), heads interleaved.
        # Scores for k-block kb+1 are emitted before AV of kb (software pipeline)
        # so the PE never waits on the exp->mask chain; h0/h1 score matmuls sit
        # at PE row-groups (0,0)/(64,0) and run concurrently in the array.
        for b in range(B):
            for j in range(4):
                qbase = 2048 * b + 512 * j
                nkb = 4 * j + 4
                avs = [psB.tile([128, 512], F32, name=f"av{h}", tag="av") for h in range(2)]
                pts = {}

                def emit_scores(kb):
                    m = kb - 4 * j
                    off = 128 * m if m >= 0 else 0
                    for h in range(2):
                        hr = 64 * h
                        ps_s = psA.tile([128, 512], F32, name="ps_s", tag="ps")
                        nc.tensor.matmul(
                            ps_s[:, off:],
                            kt_sb[hr : hr + 64, 2048 * b + 128 * kb : 2048 * b + 128 * (kb + 1)],
                            qt_sb[hr : hr + 64, qbase + off : qbase + 512],
                            start=True,
                            stop=(m < 0),
                        )
                        if m >= 0:
                            # additive causal mask on the diagonal tile:
                            # ps_s[ki, off+qi] += tri[qi, ki] (= -240 above diag)
                            nc.tensor.matmul(
                                ps_s[:, off : off + 128],
                                tri_sb[:],
                                eye_sb[:],
                                start=False,
                                stop=True,
                            )
                        pt = ptp.tile([128, 512], BF16, name="pt")
                        nc.scalar.activation(
                            pt[:, off:],
                            ps_s[:, off:],
                            mybir.ActivationFunctionType.Exp,
                            scale=SCALE,
                        )
                        pts[(kb, h)] = (pt, off)

                emit_scores(0)
                for kb in range(nkb):
                    if kb + 1 < nkb:
                        emit_scores(kb + 1)
                    for h in range(2):
                        pt, off = pts.pop((kb, h))
                        nc.tensor.matmul(
                            avs[h][0:65, off:],
                            vp_sb[:, 130 * (16 * b + kb) + 65 * h : 130 * (16 * b + kb) + 65 * h + 65],
                            pt[:, off:],
                            start=(kb == 0),
                            stop=(kb == nkb - 1),
                        )
                for h in range(2):
                    av = avs[h]
                    recip = rp.tile([1, 512], F32, name="recip")
                    nc.vector.reciprocal(recip[:], av[64:65, :])
                    rbc = rp.tile([64, 512], F32, name="rbc")
                    nc.gpsimd.partition_broadcast(rbc[:], recip[:])
                    at = atp.tile([64, 512], BF16, name="at")
                    nc.vector.tensor_mul(at[:], av[0:64, :], rbc[:])
                    s = 4 * b + j  # destination core index (token-slice owner)
                    nc.sync.dma_start(
                        a2a_in[128 * s + 64 * h : 128 * s + 64 * h + 64, :], at[:]
                    )

        # ---- phase D: reshard heads->tokens --------------------------------------
        nc.gpsimd.collective_compute(
            "AllToAll",
            mybir.AluOpType.bypass,
            replica_groups=[list(range(NC))],
            ins=[a2a_in.opt()],
            outs=[a2a_out.opt()],
        )

        # ---- phase E: output projection for my 512-token slice -------------------
        attn2 = aoutp.tile([128, 8 * 512], BF16)  # col 512c + t  (= attn^T chunks)
        for c in range(8):
            nc.sync.dma_start(
                attn2[:, 512 * c : 512 * (c + 1)],
                a2a_out[128 * c : 128 * (c + 1), :],
            )
        for mt in range(4):
            o_t = osb.tile([128, D], F32, name="o_t")
            for nh in range(2):
                ps_o = psA.tile([128, 512], F32, name="ps_o", tag="ps")
                for c in range(8):
                    nc.tensor.matmul(
                        ps_o[:],
                        attn2[:, 512 * c + 128 * mt : 512 * c + 128 * (mt + 1)],
                        wo_sb[:, 1024 * c + 512 * nh : 1024 * c + 512 * (nh + 1)],
                        start=(c == 0),
                        stop=False,
                    )
                nc.tensor.matmul(
                    ps_o[:],
                    ones_sb[:, 0:128],
                    bo_sb[:, 512 * nh : 512 * (nh + 1)],
                    start=False,
                    stop=True,
                )
                nc.vector.tensor_copy(o_t[:, 512 * nh : 512 * (nh + 1)], ps_o[:])
            nc.sync.dma_start(out[128 * mt : 128 * (mt + 1), :], o_t[:])

    nc.compile()
    return nc


_NC_CACHE = None


def _get_nc():
    global _NC_CACHE
    if _NC_CACHE is None:
        _NC_CACHE = build_nc()
    return _NC_CACHE


def _b16(a):
    return np.ascontiguousarray(np.asarray(a, np.float32).astype(ml_dtypes.bfloat16))


def make_in_maps(x, Wq, bq, Wk, bk, Wv, bv, Wo, bo):
    xf = _b16(np.asarray(x, np.float32).reshape(TI, D))
    Wq, Wk, Wv, Wo = _b16(Wq), _b16(Wk), _b16(Wv), _b16(Wo)
    bq, bk, bv, bo = _b16(bq), _b16(bk), _b16(bv), _b16(bo)
    # additive causal mask, passed pre-transposed for lhsT:
    # want ps[ki, qi] += A[ki, qi], A = 0 if ki <= qi else -240;
    # matmul adds lhsT[qi, ki] so send A^T
    A = np.where(np.arange(128)[:, None] <= np.arange(128)[None, :], 0.0, -240.0)
    tri = np.ascontiguousarray(A.T.astype(ml_dtypes.bfloat16))
    eye = np.eye(128, dtype=ml_dtypes.bfloat16)
    in_maps = []
    for r in range(NC):
        ch = slice(128 * r, 128 * (r + 1))
        in_maps.append(
            {
                "xf": xf,
                "wq": np.ascontiguousarray(Wq[:, ch]),
                "wk": np.ascontiguousarray(Wk[:, ch]),
                "wv": np.ascontiguousarray(Wv[:, ch]),
                "bq": np.ascontiguousarray(bq[ch]),
                "bk": np.ascontiguousarray(bk[ch]),
                "bv": np.ascontiguousarray(bv[ch]),
                "wo": Wo,
                "bo": bo,
                "tri": tri,
                "eye": eye,
            }
        )
    return in_maps


def assemble(results):
    out = np.empty((B, T, D), np.float32)
    for r in range(NC):
        out[r // 4, 512 * (r % 4) : 512 * (r % 4 + 1), :] = results[r]["out"]
    return out


def run(inputs, trace=False, **kw):
    nc = _get_nc()
    in_maps = make_in_maps(**inputs)
    res = run_bass_kernel_spmd(nc, in_maps, core_ids=list(range(NC)), trace=trace, **kw)
    return assemble(res.results), res


def kernel(**inputs) -> np.ndarray:
    out, _ = run(inputs)
    return out


# revision 16
# speedup vs baseline: 1.0530x; 1.0530x over previous
"""Causal self-attention (B=2, T=2048, D=1024, H=16) on 8 TRN2 NeuronCores.

Sharding: 8-way tensor-parallel over heads (2 heads/core, both batches),
then one 8-core AllToAll reshards from head-channels to token-slices so each
core computes a disjoint [512, 1024] slice of the output projection.

Per-core program (SPMD, identical program, per-core data):
  core r: heads {2r, 2r+1}  -> qkv channel slice [128r : 128r+128)
          output slice      -> batch r//4, tokens [512*(r%4), 512*(r%4)+512)

bf16 matmul inputs (host-converted), fp32 PSUM accumulation, fp32 output.
Numpy-simulated end-to-end max rel err vs the fp32 reference: ~3.3e-3.

Attention is processed as (batch, 512-query-block) groups, two groups in
flight round-robin, with both heads' scores packed into one [128,1024] PSUM
tile so each k-block costs a single Exp on ScalarE. Causality is handled by
skipping above-diagonal k-blocks plus one additive -240 mask matmul on the
diagonal tile (exp -> ~0). The softmax denominator comes from an appended
ones-column in V'; normalization is reciprocal + GpSimd partition-broadcast.
"""

import numpy as np
import ml_dtypes
from contextlib import ExitStack

import concourse.bass as bass
import concourse.tile as tile
from concourse import mybir, bacc
from concourse.bass_utils import run_bass_kernel_spmd

F32 = mybir.dt.float32
BF16 = mybir.dt.bfloat16

B, T, D, H, HD = 2, 2048, 1024, 16, 64
NC = 8  # cores
TI = B * T  # token instances = 4096
SCALE = HD ** -0.5


def build_nc() -> bass.Bass:
    nc = bacc.Bacc("TRN2", target_bir_lowering=False, debug=False, num_devices=NC)

    xf = nc.dram_tensor("xf", [TI, D], BF16, kind="ExternalInput").ap()
    wq = nc.dram_tensor("wq", [D, 128], BF16, kind="ExternalInput").ap()
    wk = nc.dram_tensor("wk", [D, 128], BF16, kind="ExternalInput").ap()
    wv = nc.dram_tensor("wv", [D, 128], BF16, kind="ExternalInput").ap()
    bq = nc.dram_tensor("bq", [128], BF16, kind="ExternalInput").ap()
    bk = nc.dram_tensor("bk", [128], BF16, kind="ExternalInput").ap()
    bv = nc.dram_tensor("bv", [128], BF16, kind="ExternalInput").ap()
    wo = nc.dram_tensor("wo", [D, D], BF16, kind="ExternalInput").ap()
    bo = nc.dram_tensor("bo", [D], BF16, kind="ExternalInput").ap()
    # additive causal mask, pre-transposed for lhsT (0 on/below diag, -240 above)
    tri = nc.dram_tensor("tri", [128, 128], BF16, kind="ExternalInput").ap()
    eye = nc.dram_tensor("eye", [128, 128], BF16, kind="ExternalInput").ap()
    out = nc.dram_tensor("out", [512, D], F32, kind="ExternalOutput").ap()

    with tile.TileContext(nc) as tc, ExitStack() as ctx:
        const = ctx.enter_context(tc.tile_pool(name="const", bufs=1))
        qkvp = ctx.enter_context(tc.tile_pool(name="qkvp", bufs=1))
        xload = ctx.enter_context(tc.tile_pool(name="xload", bufs=5))
        xtp = ctx.enter_context(tc.tile_pool(name="xtp", bufs=2))
        vtb = ctx.enter_context(tc.tile_pool(name="vtb", bufs=2))
        ptp = ctx.enter_context(tc.tile_pool(name="ptp", bufs=6))
        rp = ctx.enter_context(tc.tile_pool(name="rp", bufs=2))
        atp = ctx.enter_context(tc.tile_pool(name="atp", bufs=3))
        aoutp = ctx.enter_context(tc.tile_pool(name="aoutp", bufs=1))
        osb = ctx.enter_context(tc.tile_pool(name="osb", bufs=2))
        psS = ctx.enter_context(tc.tile_pool(name="psS", bufs=2, space="PSUM"))
        psB = ctx.enter_context(tc.tile_pool(name="psB", bufs=4, space="PSUM"))
        dram = ctx.enter_context(tc.tile_pool(name="dram", bufs=1, space="DRAM"))

        # ---- constants / weights -------------------------------------------------
        wq_sb = const.tile([128, D], BF16)  # col 128c+m  <- wq[128c+p, m]
        wk_sb = const.tile([128, D], BF16)
        wv_sb = const.tile([128, D], BF16)
        nc.sync.dma_start(
            wq_sb[:].rearrange("p (c m) -> p c m", c=8),
            wq.rearrange("(c p) m -> p c m", p=128),
        )
        nc.sync.dma_start(
            wk_sb[:].rearrange("p (c m) -> p c m", c=8),
            wk.rearrange("(c p) m -> p c m", p=128),
        )
        nc.sync.dma_start(
            wv_sb[:].rearrange("p (c m) -> p c m", c=8),
            wv.rearrange("(c p) m -> p c m", p=128),
        )
        wo_sb = const.tile([128, 8 * D], BF16)  # col 1024c+n <- wo[128c+p, n]
        nc.sync.dma_start(
            wo_sb[:].rearrange("p (c n) -> p c n", c=8),
            wo.rearrange("(c p) n -> p c n", p=128),
        )
        bq_sb = const.tile([1, 128], BF16)
        bk_sb = const.tile([1, 128], BF16)
        bv_sb = const.tile([1, 128], BF16)
        bo_sb = const.tile([1, D], BF16)
        nc.sync.dma_start(bq_sb[:], bq[None, :])
        nc.sync.dma_start(bk_sb[:], bk[None, :])
        nc.sync.dma_start(bv_sb[:], bv[None, :])
        nc.sync.dma_start(bo_sb[:], bo[None, :])
        tri_sb = const.tile([128, 128], BF16)
        eye_sb = const.tile([128, 128], BF16)
        nc.sync.dma_start(tri_sb[:], tri[:])
        nc.sync.dma_start(eye_sb[:], eye[:])
        ones_sb = const.tile([1, 512], BF16)
        nc.vector.memset(ones_sb[:], 1.0)

        # Q^T | K^T packed: col t -> Q^T, col TI + t -> K^T  (channels on partitions)
        qkt_sb = qkvp.tile([128, 2 * TI], BF16)
        # V' : [kpos(128), 32 ktiles x (2 heads x 65)]; col 130*kt + 65*h + d,
        # d==64 is the ones column (softmax denominator trick)
        vp_sb = qkvp.tile([128, 32 * 130], BF16)
        vp_ones = vp_sb.rearrange("p (kt h d) -> p kt h d", kt=32, h=2, d=65)[
            :, :, :, 64:65
        ]
        nc.vector.memset(vp_ones, 1.0)

        a2a_in = dram.tile([1024, 512], BF16)
        a2a_out = dram.tile([1024, 512], BF16)

        # ---- phase A/B: x^T then QKV projections, per 512-token block ------------
        for b in range(B):
            for blk in range(4):
                base = 2048 * b + 512 * blk
                xts = []
                for i in range(4):
                    x_t = xload.tile([128, D], BF16, name="x_t")
                    nc.sync.dma_start(x_t[:], xf[base + 128 * i : base + 128 * (i + 1), :])
                    xts.append(x_t)
                xT = xtp.tile([128, 8 * 512], BF16)  # col 512c + t
                for c2 in range(4):
                    pst = psS.tile([128, 1024], BF16, name="ps_t", tag="pss")
                    for ci in range(2):
                        c = 2 * c2 + ci
                        for i in range(4):
                            nc.tensor.transpose(
                                pst[:, 512 * ci + 128 * i : 512 * ci + 128 * (i + 1)],
                                xts[i][:, 128 * c : 128 * (c + 1)],
                                eye_sb[:],
                            )
                    nc.vector.tensor_copy(
                        xT[:, 1024 * c2 : 1024 * (c2 + 1)], pst[:]
                    )

                # Q^T and K^T share one 2-bank psum tile; single strided evacuation
                qk = psS.tile([128, 1024], F32, name="ps_qk", tag="pss")
                for half, (w_sb, b_sb) in enumerate(((wq_sb, bq_sb), (wk_sb, bk_sb))):
                    sl = slice(512 * half, 512 * (half + 1))
                    for c in range(8):
                        nc.tensor.matmul(
                            qk[:, sl],
                            w_sb[:, 128 * c : 128 * (c + 1)],
                            xT[:, 512 * c : 512 * (c + 1)],
                            start=(c == 0),
                            stop=False,
                        )
                    nc.tensor.matmul(
                        qk[:, sl], b_sb[:], ones_sb[:], start=False, stop=True
                    )
                qk_dst = qkt_sb[:].rearrange("p (s t) -> p s t", s=2)[
                    :, :, base : base + 512
                ]
                nc.scalar.copy(qk_dst, qk.rearrange("p (s t) -> p s t", s=2))

                # V^T then V' tiles via PE transpose
                vps = psS.tile([128, 512], F32, name="ps_v", tag="pss")
                for c in range(8):
                    nc.tensor.matmul(
                        vps[:],
                        wv_sb[:, 128 * c : 128 * (c + 1)],
                        xT[:, 512 * c : 512 * (c + 1)],
                        start=(c == 0),
                        stop=False,
                    )
                nc.tensor.matmul(
                    vps[:], bv_sb[:], ones_sb[:], start=False, stop=True
                )
                vt_blk = vtb.tile([128, 512], BF16)
                nc.vector.tensor_copy(vt_blk[:], vps[:])
                ps2 = psS.tile([128, 512], BF16, name="ps_vt", tag="pss")
                for i in range(4):
                    nc.tensor.transpose(
                        ps2[:, 128 * i : 128 * (i + 1)],
                        vt_blk[:, 128 * i : 128 * (i + 1)],
                        eye_sb[:],
                    )
                kt0 = 16 * b + 4 * blk
                dst = vp_sb[:, 130 * kt0 : 130 * (kt0 + 4)].rearrange(
                    "p (kt h d) -> p kt h d", kt=4, h=2, d=65
                )[:, :, :, :64]
                src = ps2.rearrange("p (i h d) -> p i h d", i=4, h=2, d=64)
                nc.vector.tensor_copy(dst, src)

        # ---- phase C: attention; two (batch, q-block) groups in flight -----------
        def kq(sl_base, lo, hi):
            return qkt_sb[:, sl_base + lo : sl_base + hi]

        class Group:
            def __init__(self, b, j):
                self.b, self.j = b, j
                self.nkb = 4 * j + 4
                self.kb_s = 0  # next k-block to score
                self.kb_a = 0  # next k-block to accumulate into AV
                self.qbase = 2048 * b + 512 * j
                self.avs = [
                    psB.tile([128, 512], F32, name=f"av{h}", tag="av")
                    for h in range(2)
                ]
                self.pts = {}

            def emit_scores(self):
                kb = self.kb_s
                self.kb_s += 1
                m = kb - 4 * self.j
                off = 128 * m if m >= 0 else 0
                ps_s = psS.tile([128, 1024], F32, name="ps_s", tag="pss")
                for h in range(2):
                    hr = 64 * h
                    hb = 512 * h
                    nc.tensor.matmul(
                        ps_s[:, hb + off : hb + 512],
                        qkt_sb[
                            hr : hr + 64,
                            TI + 2048 * self.b + 128 * kb : TI + 2048 * self.b + 128 * (kb + 1),
                        ],
                        qkt_sb[hr : hr + 64, self.qbase + off : self.qbase + 512],
                        start=True,
                        stop=(m < 0),
                    )
                    if m >= 0:
                        # additive causal mask: ps += tri[qi, ki] (-240 above diag)
                        nc.tensor.matmul(
                            ps_s[:, hb + off : hb + off + 128],
                            tri_sb[:],
                            eye_sb[:],
                            start=False,
                            stop=True,
                        )
                pt = ptp.tile([128, 1024], BF16, name="pt")
                pt_v = pt.rearrange("p (s t) -> p s t", s=2)[:, :, off:512]
                ps_v = ps_s.rearrange("p (s t) -> p s t", s=2)[:, :, off:512]
                nc.scalar.activation(
                    pt_v, ps_v, mybir.ActivationFunctionType.Exp, scale=SCALE
                )
                self.pts[kb] = (pt, off)

            def emit_av(self):
                kb = self.kb_a
                self.kb_a += 1
                pt, off = self.pts.pop(kb)
                for h in range(2):
                    nc.tensor.matmul(
                        self.avs[h][0:65, off:],
                        vp_sb[
                            :,
                            130 * (16 * self.b + kb) + 65 * h : 130 * (16 * self.b + kb) + 65 * h + 65,
                        ],
                        pt[:, 512 * h + off : 512 * (h + 1)],
                        start=(kb == 0),
                        stop=(kb == self.nkb - 1),
                    )

            def finalize(self):
                s = 4 * self.b + self.j  # destination core (token-slice owner)
                for h in range(2):
                    rec = rp.tile([1, 512], F32, name="rec")
                    nc.vector.reciprocal(rec[:], self.avs[h][64:65, :])
                    rbc = rp.tile([64, 512], F32, name="rbc")
                    nc.gpsimd.partition_broadcast(rbc[:], rec[:])
                    at = atp.tile([64, 512], BF16, name="at")
                    nc.vector.tensor_mul(at[:], self.avs[h][0:64, :], rbc[:])
                    nc.sync.dma_start(
                        a2a_in[128 * s + 64 * h : 128 * s + 64 * h + 64, :], at[:]
                    )

        # long groups first so two groups stay in flight most of the time
        queue = [(0, 3), (0, 2), (0, 1), (0, 0), (1, 3), (1, 2), (1, 1), (1, 0)]
        active = []
        while queue or active:
            while len(active) < 2 and queue:
                g = Group(*queue.pop(0))
                g.emit_scores()
                active.append(g)
            for g in list(active):
                if g.kb_s < g.nkb:
                    g.emit_scores()
                if g.kb_a < g.kb_s:
                    g.emit_av()
                if g.kb_a == g.nkb:
                    g.finalize()
                    active.remove(g)

        # ---- phase D: reshard heads->tokens --------------------------------------
        nc.gpsimd.collective_compute(
            "AllToAll",
            mybir.AluOpType.bypass,
            replica_groups=[list(range(NC))],
            ins=[a2a_in.opt()],
            outs=[a2a_out.opt()],
        )

        # ---- phase E: output projection for my 512-token slice -------------------
        attn2 = aoutp.tile([128, 8 * 512], BF16)  # col 512c + t  (= attn^T chunks)
        for c in range(8):
            nc.sync.dma_start(
                attn2[:, 512 * c : 512 * (c + 1)],
                a2a_out[128 * c : 128 * (c + 1), :],
            )
        for mt in range(4):
            po = psS.tile([128, 1024], F32, name="ps_o", tag="pss")
            for nh in range(2):
                sl = slice(512 * nh, 512 * (nh + 1))
                for c in range(8):
                    nc.tensor.matmul(
                        po[:, sl],
                        attn2[:, 512 * c + 128 * mt : 512 * c + 128 * (mt + 1)],
                        wo_sb[:, 1024 * c + 512 * nh : 1024 * c + 512 * (nh + 1)],
                        start=(c == 0),
                        stop=False,
                    )
                nc.tensor.matmul(
                    po[:, sl],
                    ones_sb[:, 0:128],
                    bo_sb[:, 512 * nh : 512 * (nh + 1)],
                    start=False,
                    stop=True,
                )
            o_t = osb.tile([128, D], F32, name="o_t")
            nc.vector.tensor_copy(o_t[:], po[:])
            nc.sync.dma_start(out[128 * mt : 128 * (mt + 1), :], o_t[:])

    nc.compile()
    return nc


_NC_CACHE = None


def _get_nc():
    global _NC_CACHE
    if _NC_CACHE is None:
        _NC_CACHE = build_nc()
    return _NC_CACHE


def _b16(a):
    return np.ascontiguousarray(np.asarray(a, np.float32).astype(ml_dtypes.bfloat16))


def make_in_maps(x, Wq, bq, Wk, bk, Wv, bv, Wo, bo):
    xf = _b16(np.asarray(x, np.float32).reshape(TI, D))
    Wq, Wk, Wv, Wo = _b16(Wq), _b16(Wk), _b16(Wv), _b16(Wo)
    bq, bk, bv, bo = _b16(bq), _b16(bk), _b16(bv), _b16(bo)
    # additive causal mask, passed pre-transposed for lhsT:
    # want ps[ki, qi] += A[ki, qi], A = 0 if ki <= qi else -240;
    # matmul adds lhsT[qi, ki] so send A^T
    A = np.where(np.arange(128)[:, None] <= np.arange(128)[None, :], 0.0, -240.0)
    tri = np.ascontiguousarray(A.T.astype(ml_dtypes.bfloat16))
    eye = np.eye(128, dtype=ml_dtypes.bfloat16)
    in_maps = []
    for r in range(NC):
        ch = slice(128 * r, 128 * (r + 1))
        in_maps.append(
            {
                "xf": xf,
                "wq": np.ascontiguousarray(Wq[:, ch]),
                "wk": np.ascontiguousarray(Wk[:, ch]),
                "wv": np.ascontiguousarray(Wv[:, ch]),
                "bq": np.ascontiguousarray(bq[ch]),
                "bk": np.ascontiguousarray(bk[ch]),
                "bv": np.ascontiguousarray(bv[ch]),
                "wo": Wo,
                "bo": bo,
                "tri": tri,
                "eye": eye,
            }
        )
    return in_maps


def assemble(results):
    out = np.empty((B, T, D), np.float32)
    for r in range(NC):
        out[r // 4, 512 * (r % 4) : 512 * (r % 4 + 1), :] = results[r]["out"]
    return out


def run(inputs, trace=False, **kw):
    nc = _get_nc()
    in_maps = make_in_maps(**inputs)
    res = run_bass_kernel_spmd(nc, in_maps, core_ids=list(range(NC)), trace=trace, **kw)
    return assemble(res.results), res


def kernel(**inputs) -> np.ndarray:
    out, _ = run(inputs)
    return out


# revision 19
# speedup vs baseline: 1.1497x; 1.0918x over previous
"""Causal self-attention (B=2, T=2048, D=1024, H=16) on 8 TRN2 NeuronCores.

Sharding: 8-way tensor-parallel over heads (2 heads/core, both batches),
then one 8-core AllToAll reshards from head-channels to token-slices so each
core computes a disjoint [512, 1024] slice of the output projection.

Per-core program (SPMD, identical program, per-core data):
  core r: heads {2r, 2r+1}  -> qkv channel slice [128r : 128r+128)
          output slice      -> batch r//4, tokens [512*(r%4), 512*(r%4)+512)

bf16 matmul inputs (host-converted), fp32 PSUM accumulation, fp32 output.
Numpy-simulated end-to-end max rel err vs the fp32 reference: ~3.3e-3.

Attention is processed as (batch, 512-query-block) groups, two groups in
flight round-robin, with both heads' scores packed into one [128,1024] PSUM
tile so each k-block costs a single Exp on ScalarE. Causality is handled by
skipping above-diagonal k-blocks plus one additive -240 mask matmul on the
diagonal tile (exp -> ~0). The softmax denominator comes from an appended
ones-column in V'; normalization is reciprocal + GpSimd partition-broadcast.
"""

import numpy as np
import ml_dtypes
from contextlib import ExitStack

import concourse.bass as bass
import concourse.tile as tile
from concourse import mybir, bacc
from concourse.bass_utils import run_bass_kernel_spmd

F32 = mybir.dt.float32
BF16 = mybir.dt.bfloat16

B, T, D, H, HD = 2, 2048, 1024, 16, 64
NC = 8  # cores
TI = B * T  # token instances = 4096
SCALE = HD ** -0.5


def build_nc() -> bass.Bass:
    nc = bacc.Bacc("TRN2", target_bir_lowering=False, debug=False, num_devices=NC)

    xf = nc.dram_tensor("xf", [TI, D], BF16, kind="ExternalInput").ap()
    wq = nc.dram_tensor("wq", [D, 128], BF16, kind="ExternalInput").ap()
    wk = nc.dram_tensor("wk", [D, 128], BF16, kind="ExternalInput").ap()
    wv = nc.dram_tensor("wv", [D, 128], BF16, kind="ExternalInput").ap()
    bq = nc.dram_tensor("bq", [128], BF16, kind="ExternalInput").ap()
    bk = nc.dram_tensor("bk", [128], BF16, kind="ExternalInput").ap()
    bv = nc.dram_tensor("bv", [128], BF16, kind="ExternalInput").ap()
    wo = nc.dram_tensor("wo", [D, D], BF16, kind="ExternalInput").ap()
    bo = nc.dram_tensor("bo", [D], BF16, kind="ExternalInput").ap()
    # additive causal mask, pre-transposed for lhsT (0 on/below diag, -240 above)
    tri = nc.dram_tensor("tri", [128, 128], BF16, kind="ExternalInput").ap()
    eye = nc.dram_tensor("eye", [128, 128], BF16, kind="ExternalInput").ap()
    out = nc.dram_tensor("out", [512, D], F32, kind="ExternalOutput").ap()

    with tile.TileContext(nc) as tc, ExitStack() as ctx:
        const = ctx.enter_context(tc.tile_pool(name="const", bufs=1))
        qkvp = ctx.enter_context(tc.tile_pool(name="qkvp", bufs=1))
        xload = ctx.enter_context(tc.tile_pool(name="xload", bufs=5))
        xtp = ctx.enter_context(tc.tile_pool(name="xtp", bufs=2))
        vtb = ctx.enter_context(tc.tile_pool(name="vtb", bufs=2))
        ptp = ctx.enter_context(tc.tile_pool(name="ptp", bufs=6))
        rp = ctx.enter_context(tc.tile_pool(name="rp", bufs=2))
        atp = ctx.enter_context(tc.tile_pool(name="atp", bufs=3))
        aoutp = ctx.enter_context(tc.tile_pool(name="aoutp", bufs=1))
        osb = ctx.enter_context(tc.tile_pool(name="osb", bufs=2))
        psS = ctx.enter_context(tc.tile_pool(name="psS", bufs=2, space="PSUM"))
        psB = ctx.enter_context(tc.tile_pool(name="psB", bufs=4, space="PSUM"))
        dram = ctx.enter_context(tc.tile_pool(name="dram", bufs=1, space="DRAM"))

        # ---- constants / weights -------------------------------------------------
        wq_sb = const.tile([128, D], BF16)  # col 128c+m  <- wq[128c+p, m]
        wk_sb = const.tile([128, D], BF16)
        wv_sb = const.tile([128, D], BF16)
        nc.sync.dma_start(
            wq_sb[:].rearrange("p (c m) -> p c m", c=8),
            wq.rearrange("(c p) m -> p c m", p=128),
        )
        nc.sync.dma_start(
            wk_sb[:].rearrange("p (c m) -> p c m", c=8),
            wk.rearrange("(c p) m -> p c m", p=128),
        )
        nc.sync.dma_start(
            wv_sb[:].rearrange("p (c m) -> p c m", c=8),
            wv.rearrange("(c p) m -> p c m", p=128),
        )
        wo_sb = const.tile([128, 8 * D], BF16)  # col 1024c+n <- wo[128c+p, n]
        nc.sync.dma_start(
            wo_sb[:].rearrange("p (c n) -> p c n", c=8),
            wo.rearrange("(c p) n -> p c n", p=128),
        )
        bq_sb = const.tile([1, 128], BF16)
        bk_sb = const.tile([1, 128], BF16)
        bv_sb = const.tile([1, 128], BF16)
        bo_sb = const.tile([1, D], BF16)
        nc.sync.dma_start(bq_sb[:], bq[None, :])
        nc.sync.dma_start(bk_sb[:], bk[None, :])
        nc.sync.dma_start(bv_sb[:], bv[None, :])
        nc.sync.dma_start(bo_sb[:], bo[None, :])
        tri_sb = const.tile([128, 128], BF16)
        eye_sb = const.tile([128, 128], BF16)
        nc.sync.dma_start(tri_sb[:], tri[:])
        nc.sync.dma_start(eye_sb[:], eye[:])
        ones_sb = const.tile([1, 512], BF16)
        nc.vector.memset(ones_sb[:], 1.0)

        # Q^T | K^T packed: col t -> Q^T, col TI + t -> K^T  (channels on partitions)
        qkt_sb = qkvp.tile([128, 2 * TI], BF16)
        # V' : [kpos(128), 32 ktiles x (2 heads x 65)]; col 130*kt + 65*h + d,
        # d==64 is the ones column (softmax denominator trick)
        vp_sb = qkvp.tile([128, 32 * 130], BF16)
        vp_ones = vp_sb.rearrange("p (kt h d) -> p kt h d", kt=32, h=2, d=65)[
            :, :, :, 64:65
        ]
        nc.vector.memset(vp_ones, 1.0)

        a2a_in = dram.tile([1024, 512], BF16)
        a2a_out = dram.tile([1024, 512], BF16)

        # ---- phase A/B: x^T then QKV projections, per 512-token block ------------
        for b in range(B):
            for blk in range(4):
                base = 2048 * b + 512 * blk
                xts = []
                for i in range(4):
                    x_t = xload.tile([128, D], BF16, name="x_t")
                    nc.sync.dma_start(x_t[:], xf[base + 128 * i : base + 128 * (i + 1), :])
                    xts.append(x_t)
                xT = xtp.tile([128, 8 * 512], BF16)  # col 512c + t
                for c2 in range(4):
                    pst = psS.tile([128, 1024], BF16, name="ps_t", tag="pss")
                    for ci in range(2):
                        c = 2 * c2 + ci
                        for i in range(4):
                            nc.tensor.transpose(
                                pst[:, 512 * ci + 128 * i : 512 * ci + 128 * (i + 1)],
                                xts[i][:, 128 * c : 128 * (c + 1)],
                                eye_sb[:],
                            )
                    nc.vector.tensor_copy(
                        xT[:, 1024 * c2 : 1024 * (c2 + 1)], pst[:]
                    )

                # Q^T and K^T share one 2-bank psum tile; single strided evacuation
                qk = psS.tile([128, 1024], F32, name="ps_qk", tag="pss")
                for half, (w_sb, b_sb) in enumerate(((wq_sb, bq_sb), (wk_sb, bk_sb))):
                    sl = slice(512 * half, 512 * (half + 1))
                    for c in range(8):
                        nc.tensor.matmul(
                            qk[:, sl],
                            w_sb[:, 128 * c : 128 * (c + 1)],
                            xT[:, 512 * c : 512 * (c + 1)],
                            start=(c == 0),
                            stop=False,
                        )
                    nc.tensor.matmul(
                        qk[:, sl], b_sb[:], ones_sb[:], start=False, stop=True
                    )
                qk_dst = qkt_sb[:].rearrange("p (s t) -> p s t", s=2)[
                    :, :, base : base + 512
                ]
                nc.scalar.copy(qk_dst, qk.rearrange("p (s t) -> p s t", s=2))

                # V^T then V' tiles via PE transpose
                vps = psS.tile([128, 512], F32, name="ps_v", tag="pss")
                for c in range(8):
                    nc.tensor.matmul(
                        vps[:],
                        wv_sb[:, 128 * c : 128 * (c + 1)],
                        xT[:, 512 * c : 512 * (c + 1)],
                        start=(c == 0),
                        stop=False,
                    )
                nc.tensor.matmul(
                    vps[:], bv_sb[:], ones_sb[:], start=False, stop=True
                )
                vt_blk = vtb.tile([128, 512], BF16)
                nc.vector.tensor_copy(vt_blk[:], vps[:])
                ps2 = psS.tile([128, 512], BF16, name="ps_vt", tag="pss")
                for i in range(4):
                    nc.tensor.transpose(
                        ps2[:, 128 * i : 128 * (i + 1)],
                        vt_blk[:, 128 * i : 128 * (i + 1)],
                        eye_sb[:],
                    )
                kt0 = 16 * b + 4 * blk
                dst = vp_sb[:, 130 * kt0 : 130 * (kt0 + 4)].rearrange(
                    "p (kt h d) -> p kt h d", kt=4, h=2, d=65
                )[:, :, :, :64]
                src = ps2.rearrange("p (i h d) -> p i h d", i=4, h=2, d=64)
                nc.vector.tensor_copy(dst, src)

        # ---- phase C: attention; two (batch, q-block) groups in flight -----------
        def kq(sl_base, lo, hi):
            return qkt_sb[:, sl_base + lo : sl_base + hi]

        class Group:
            def __init__(self, b, j):
                self.b, self.j = b, j
                self.nkb = 4 * j + 4
                self.kb_s = 0  # next k-block to score
                self.kb_a = 0  # next k-block to accumulate into AV
                self.qbase = 2048 * b + 512 * j
                self.avs = [
                    psB.tile([128, 512], F32, name=f"av{h}", tag="av")
                    for h in range(2)
                ]
                self.pts = {}

            def emit_scores(self):
                kb = self.kb_s
                self.kb_s += 1
                m = kb - 4 * self.j
                off = 128 * m if m >= 0 else 0
                ps_s = psS.tile([128, 1024], F32, name="ps_s", tag="pss")
                for h in range(2):
                    hr = 64 * h
                    hb = 512 * h
                    nc.tensor.matmul(
                        ps_s[:, hb + off : hb + 512],
                        qkt_sb[
                            hr : hr + 64,
                            TI + 2048 * self.b + 128 * kb : TI + 2048 * self.b + 128 * (kb + 1),
                        ],
                        qkt_sb[hr : hr + 64, self.qbase + off : self.qbase + 512],
                        start=True,
                        stop=(m < 0),
                    )
                    if m >= 0:
                        # additive causal mask: ps += tri[qi, ki] (-240 above diag)
                        nc.tensor.matmul(
                            ps_s[:, hb + off : hb + off + 128],
                            tri_sb[:],
                            eye_sb[:],
                            start=False,
                            stop=True,
                        )
                pt = ptp.tile([128, 1024], BF16, name="pt")
                pt_v = pt.rearrange("p (s t) -> p s t", s=2)[:, :, off:512]
                ps_v = ps_s.rearrange("p (s t) -> p s t", s=2)[:, :, off:512]
                nc.scalar.activation(
                    pt_v, ps_v, mybir.ActivationFunctionType.Exp, scale=SCALE
                )
                self.pts[kb] = (pt, off)

            def emit_av(self):
                kb = self.kb_a
                self.kb_a += 1
                pt, off = self.pts.pop(kb)
                for h in range(2):
                    nc.tensor.matmul(
                        self.avs[h][0:65, off:],
                        vp_sb[
                            :,
                            130 * (16 * self.b + kb) + 65 * h : 130 * (16 * self.b + kb) + 65 * h + 65,
                        ],
                        pt[:, 512 * h + off : 512 * (h + 1)],
                        start=(kb == 0),
                        stop=(kb == self.nkb - 1),
                    )

            def finalize(self):
                s = 4 * self.b + self.j  # destination core (token-slice owner)
                for h in range(2):
                    rec = rp.tile([1, 512], F32, name="rec")
                    nc.vector.reciprocal(rec[:], self.avs[h][64:65, :])
                    rbc = rp.tile([64, 512], F32, name="rbc")
                    nc.gpsimd.partition_broadcast(rbc[:], rec[:])
                    at = atp.tile([64, 512], BF16, name="at")
                    nc.vector.tensor_mul(at[:], self.avs[h][0:64, :], rbc[:])
                    nc.sync.dma_start(
                        a2a_in[128 * s + 64 * h : 128 * s + 64 * h + 64, :], at[:]
                    )

        # long groups first so two groups stay in flight most of the time
        queue = [(0, 3), (0, 2), (0, 1), (0, 0), (1, 3), (1, 2), (1, 1), (1, 0)]
        active = []
        while queue or active:
            while len(active) < 2 and queue:
                g = Group(*queue.pop(0))
                g.emit_scores()
                active.append(g)
            for g in list(active):
                if g.kb_s < g.nkb:
                    g.emit_scores()
                if g.kb_a < g.kb_s:
                    g.emit_av()
                if g.kb_a == g.nkb:
                    g.finalize()
                    active.remove(g)

        # ---- phase D: reshard heads->tokens --------------------------------------
        nc.gpsimd.collective_compute(
            "AllToAll",
            mybir.AluOpType.bypass,
            replica_groups=[list(range(NC))],
            ins=[a2a_in.opt()],
            outs=[a2a_out.opt()],
        )

        # ---- phase E: output projection for my 512-token slice -------------------
        attn2 = aoutp.tile([128, 8 * 512], BF16)  # col 512c + t  (= attn^T chunks)
        for c in range(8):
            nc.sync.dma_start(
                attn2[:, 512 * c : 512 * (c + 1)],
                a2a_out[128 * c : 128 * (c + 1), :],
            )
        for mt in range(4):
            po = psS.tile([128, 1024], F32, name="ps_o", tag="pss")
            for nh in range(2):
                sl = slice(512 * nh, 512 * (nh + 1))
                for c in range(8):
                    nc.tensor.matmul(
                        po[:, sl],
                        attn2[:, 512 * c + 128 * mt : 512 * c + 128 * (mt + 1)],
                        wo_sb[:, 1024 * c + 512 * nh : 1024 * c + 512 * (nh + 1)],
                        start=(c == 0),
                        stop=False,
                    )
                nc.tensor.matmul(
                    po[:, sl],
                    ones_sb[:, 0:128],
                    bo_sb[:, 512 * nh : 512 * (nh + 1)],
                    start=False,
                    stop=True,
                )
            o_t = osb.tile([128, D], F32, name="o_t")
            nc.vector.tensor_copy(o_t[:], po[:])
            nc.sync.dma_start(out[128 * mt : 128 * (mt + 1), :], o_t[:])

    nc.compile()
    return nc


_NC_CACHE = None


def _get_nc():
    global _NC_CACHE
    if _NC_CACHE is None:
        _NC_CACHE = build_nc()
    return _NC_CACHE


def _b16(a):
    return np.ascontiguousarray(np.asarray(a, np.float32).astype(ml_dtypes.bfloat16))


def make_in_maps(x, Wq, bq, Wk, bk, Wv, bv, Wo, bo):
    xf = _b16(np.asarray(x, np.float32).reshape(TI, D))
    Wq, Wk, Wv, Wo = _b16(Wq), _b16(Wk), _b16(Wv), _b16(Wo)
    bq, bk, bv, bo = _b16(bq), _b16(bk), _b16(bv), _b16(bo)
    # additive causal mask, passed pre-transposed for lhsT:
    # want ps[ki, qi] += A[ki, qi], A = 0 if ki <= qi else -240;
    # matmul adds lhsT[qi, ki] so send A^T
    A = np.where(np.arange(128)[:, None] <= np.arange(128)[None, :], 0.0, -240.0)
    tri = np.ascontiguousarray(A.T.astype(ml_dtypes.bfloat16))
    eye = np.eye(128, dtype=ml_dtypes.bfloat16)
    in_maps = []
    for r in range(NC):
        ch = slice(128 * r, 128 * (r + 1))
        in_maps.append(
            {
                "xf": xf,
                "wq": np.ascontiguousarray(Wq[:, ch]),
                "wk": np.ascontiguousarray(Wk[:, ch]),
                "wv": np.ascontiguousarray(Wv[:, ch]),
                "bq": np.ascontiguousarray(bq[ch]),
                "bk": np.ascontiguousarray(bk[ch]),
                "bv": np.ascontiguousarray(bv[ch]),
                "wo": Wo,
                "bo": bo,
                "tri": tri,
                "eye": eye,
            }
        )
    return in_maps


def assemble(results):
    out = np.empty((B, T, D), np.float32)
    for r in range(NC):
        out[r // 4, 512 * (r % 4) : 512 * (r % 4 + 1), :] = results[r]["out"]
    return out


def run(inputs, trace=False, **kw):
    nc = _get_nc()
    in_maps = make_in_maps(**inputs)
    res = run_bass_kernel_spmd(nc, in_maps, core_ids=list(range(NC)), trace=trace, **kw)
    return assemble(res.results), res


def kernel(**inputs) -> np.ndarray:
    out, _ = run(inputs)
    return out
